# revision 13
# baseline (speedup 1.0000x reference)
"""Causal single-head attention (B=256, T=256, C=384, H=64) on 8 trn2 cores.

Data-parallel over batch: each core computes bpc=32 batches independently.

v2 design (vs baseline): fp16 matmul dtype (1 cycle/row at any output size,
halves DMA bytes), transposed-softmax formulation (weiT = k^T q with s on
partitions -> no PE transposes at all), natural-layout v projection
(x-stationary), row sums via an appended ones-column in the AV matmul,
causal masking as a multiplicative 0/1 triangle on the Pool engine (SBUF
fp16), and group-batched DMAs (G batches per DMA) to amortize per-DMA
overheads (~625ns HWDGE + ~565ns SEQ each).

Per batch:
  qT[h,t], kT[h,t] = Wq/Wk stationary @ xT moving     (2x3 matmuls, [64,2,256] PSUM)
  v[t,h]           = xT-block stationary @ Wv moving  (6 matmuls, [128,2,64] PSUM)
  weiT[s,t]        = kT-block stationary @ qT moving  (2 matmuls, [128,384] PSUM:
                     cols 0:256 = s0 x all t, cols 256:384 = s1 x t1)
  p = exp(weiT) on ACT (PSUM->SBUF fp16, one instruction; logits ~N(0,1) so no
      max-subtraction needed), diagonal blocks masked by 0/1 upper-triangle
      multiply on Pool.
  out[t, 0:64] + rowsum[t] = p-block stationary @ [v|1] moving (3 matmuls)
  out scaled by 1/rowsum on the PSUM->SBUF copy (DVE for t0, ACT for t1).
"""

import contextlib
import os
import sys

import numpy as np

for _p in ("/opt/trn_rl_repo",):
    if _p not in sys.path:
        sys.path.insert(0, _p)

B, T, C, H = 256, 256, 384, 64
N_CORES = 8
BPC = B // N_CORES  # batches per core
P = 128

LAST_RESULT = None  # BassKernelResults of the most recent run (for test.py)


def _build_nc(bpc=BPC, repeats=1, group=8):
    import concourse.bacc as bacc
    import concourse.mybir as mybir
    import concourse.tile as tile
    from concourse.masks import make_upper_triangular

    f32 = mybir.dt.float32
    f16 = mybir.dt.float16

    G = min(group, bpc)
    assert bpc % G == 0
    NG = bpc // G

    nc = bacc.Bacc("TRN2", target_bir_lowering=False, debug=False)

    xh = nc.dram_tensor("xh", [3, P, bpc, T], f16, kind="ExternalInput")
    wqk = nc.dram_tensor("wqk", [P, 3, P], f16, kind="ExternalInput")
    wv = nc.dram_tensor("wv", [P, 3, H], f16, kind="ExternalInput")
    oh = nc.dram_tensor("oh", [P, bpc, 2, H], f16, kind="ExternalOutput")

    Exp = mybir.ActivationFunctionType.Exp
    Copy = mybir.ActivationFunctionType.Copy
    mult = mybir.AluOpType.mult

    xh_r = xh.rearrange("c p b t -> p c b t")

    with tile.TileContext(nc) as tc:
        with (
            tc.tile_pool(name="consts", bufs=1) as consts,
            tc.tile_pool(name="xg", bufs=2) as xg_pool,
            tc.tile_pool(name="og", bufs=2) as og_pool,
            tc.tile_pool(name="sb", bufs=8) as sb,
            tc.tile_pool(name="ps_a", bufs=3, space="PSUM") as ps_a,
            tc.tile_pool(name="ps_wei", bufs=3, space="PSUM") as ps_wei,
            tc.tile_pool(name="ps_o", bufs=2, space="PSUM") as ps_o,
        ):
            wqk_sb = consts.tile([P, 3, P], f16)
            nc.sync.dma_start(wqk_sb, wqk[:])
            wv_sb = consts.tile([P, 3, H], f16)
            nc.sync.dma_start(wv_sb, wv[:])
            tri01 = consts.tile([P, P], f16)
            make_upper_triangular(nc, tri01, val=1.0, diag=True)

            rep_ctx = (
                tc.For_i(0, repeats, 1, hint_engines=(mybir.EngineType.PE,
                                                      mybir.EngineType.DVE,
                                                      mybir.EngineType.Activation,
                                                      mybir.EngineType.Pool,
                                                      mybir.EngineType.SP))
                if repeats > 1
                else contextlib.nullcontext()
            )
            with rep_ctx:
              def emit_tail(pv):
                  """Deferred AV (PE) for batch pv, emitted one batch later."""
                  o_t = ps_o.tile([P, 2, H + 1], f32, tag="o", name="o")
                  pv["o_ps"] = o_t
                  p_sb, v_aug = pv["p"], pv["v_aug"]
                  nc.tensor.matmul(
                      o_t[:, 0, :], p_sb[:, 0:P], v_aug[:, 0, :],
                      start=True, stop=True,
                  )
                  nc.tensor.matmul(
                      o_t[:, 1, :], p_sb[:, P:T], v_aug[:, 0, :],
                      start=True, stop=False,
                  )
                  nc.tensor.matmul(
                      o_t[:, 1, :], p_sb[:, T:3 * P], v_aug[:, 1, :],
                      start=False, stop=True,
                  )

              def emit_norm(pv):
                  o_t = pv["o_ps"]
                  rinv = sb.tile([P, 2], f32, tag="rinv", name="rinv")
                  nc.vector.reciprocal(rinv, o_t[:, :, H])
                  nc.vector.tensor_tensor(
                      pv["og"][:, pv["j"], :, :], o_t[:, :, 0:H],
                      rinv[:, :, None].to_broadcast((P, 2, H)), mult,
                  )
                  if pv["last_in_group"]:
                      nc.sync.dma_start(
                          oh[:, pv["g"] * G:(pv["g"] + 1) * G, :, :], pv["og"])

              prev = None
              cur = {}
              for idx in range(bpc):
                g, j = divmod(idx, G)
                if j == 0:
                    cur["xg"] = xg_pool.tile(
                        [P, 3, G, T], f16, tag="xg", name="xg")
                    nc.sync.dma_start(
                        cur["xg"], xh_r[:, :, g * G:(g + 1) * G, :])
                    cur["og"] = og_pool.tile(
                        [P, G, 2, H], f16, tag="og", name="og")
                xg, og = cur["xg"], cur["og"]
                if True:
                    # ---- projections. q|k packed in one 128-wide stationary:
                    # qT lands on PSUM partitions 0:64, kT on 64:128. qk and v
                    # share one PSUM bank (qk bytes 0:1024, v 1024:1536);
                    # their accumulation groups open sequentially in PE
                    # program order and PSUM zeroing is lazy per byte. ----
                    bkA = ps_a.tile([P, 384], f32, tag="bkA")
                    qk_ps = bkA[:, 0:T]
                    v_ps = bkA[:, T:T + P].rearrange("p (i h) -> p i h", i=2)
                    for c in range(3):
                        nc.tensor.matmul(
                            qk_ps, wqk_sb[:, c, :], xg[:, c, j, :],
                            start=(c == 0), stop=(c == 2),
                        )
                    for i in range(2):
                        for c in range(3):
                            nc.tensor.matmul(
                                v_ps[:, i, :],
                                xg[:, c, j, i * P:(i + 1) * P], wv_sb[:, c, :],
                                start=(c == 0), stop=(c == 2),
                            )
                    # deferred AV of batch idx-1 fills the PE gap while this
                    # batch's q/k copies land
                    if prev is not None:
                        emit_tail(prev)

                    # qT (PSUM partitions 0:64) and kT (64:128) both land on
                    # SBUF partitions 0:64; the k copy crosses partitions
                    # (validated on HW) so the weiT matmul sees both operands
                    # at base partition 0.
                    qk_sb = sb.tile([H, 2, T], f16, tag="qk_sb")
                    nc.scalar.copy(qk_sb[:, 0, :], qk_ps[0:H, :])
                    nc.vector.tensor_copy(qk_sb[:, 1, :], qk_ps[H:P, :])
                    v_aug = sb.tile([P, 2, H + 1], f16, tag="v_aug")
                    nc.gpsimd.memset(v_aug[:, :, H:H + 1], 1.0)
                    nc.vector.tensor_copy(v_aug[:, :, 0:H], v_ps)

                    # deferred normalization of batch idx-1 (DVE/SP)
                    if prev is not None:
                        emit_norm(prev)

                    # ---- weiT = k^T q, [s, t] with s on partitions ----
                    wei_ps = ps_wei.tile([P, 3 * P], f32, tag="wei")
                    nc.tensor.matmul(
                        wei_ps[:, 0:T], qk_sb[:, 1, 0:P], qk_sb[:, 0, :],
                        start=True, stop=True,
                    )
                    nc.tensor.matmul(
                        wei_ps[:, T:3 * P], qk_sb[:, 1, P:T], qk_sb[:, 0, P:T],
                        start=True, stop=True,
                    )

                    # ---- softmax numerator (no max subtraction) ----
                    p_sb = sb.tile([P, 3 * P], f16, tag="p")
                    nc.scalar.activation(p_sb, wei_ps, Exp)
                    # causal mask: zero strict-lower triangle of the two
                    # diagonal (s,t) blocks (cols 0:128 and 256:384) in one
                    # strided op
                    p_diag = p_sb.rearrange("p (a q) -> p a q", q=P)[:, 0::2, :]
                    nc.gpsimd.tensor_tensor(
                        p_diag, p_diag,
                        tri01[:, None, :].to_broadcast((P, 2, P)), mult,
                    )

                    prev = {
                        "p": p_sb, "v_aug": v_aug, "og": og, "j": j, "g": g,
                        "last_in_group": j == G - 1,
                    }

              # drain the final batch
              emit_tail(prev)
              emit_norm(prev)

    nc.compile()
    return nc


def _prep_inputs(x, Wk, Wq, Wv):
    """Full inputs -> per-core in_maps with the DRAM layouts above."""
    x = np.asarray(x, dtype=np.float32)
    scale = np.float32(H) ** np.float32(-0.5)
    wq = np.asarray(Wq, dtype=np.float32) * scale
    wk = np.asarray(Wk, dtype=np.float32)
    wv = np.asarray(Wv, dtype=np.float32)
    # wqk[p, c, 0:64] = Wq_scaled, wqk[p, c, 64:128] = Wk
    wqk_arr = np.concatenate(
        [wq.reshape(3, P, H), wk.reshape(3, P, H)], axis=2
    ).transpose(1, 0, 2)
    wqk_arr = np.ascontiguousarray(wqk_arr.astype(np.float16))
    wv_arr = np.ascontiguousarray(
        wv.reshape(3, P, H).transpose(1, 0, 2).astype(np.float16)
    )
    in_maps = []
    for cid in range(N_CORES):
        xc = x[cid * BPC:(cid + 1) * BPC]  # [bpc, T, C]
        xh = xc.reshape(BPC, T, 3, P).transpose(2, 3, 0, 1)  # [3, P, bpc, T]
        in_maps.append({
            "xh": np.ascontiguousarray(xh.astype(np.float16)),
            "wqk": wqk_arr,
            "wv": wv_arr,
        })
    return in_maps


def _assemble_output(results):
    """Per-core oh [P, bpc, 2, H] fp16 -> full out [B, T, H] fp32."""
    outs = []
    for r in results:
        oh = np.asarray(r["oh"], dtype=np.float32)  # [P, bpc, 2, H]
        outs.append(oh.transpose(1, 2, 0, 3).reshape(BPC, T, H))
    return np.concatenate(outs, axis=0)


def kernel(x, Wk, Wq, Wv):
    global LAST_RESULT
    from concourse.bass_utils import run_bass_kernel_spmd

    in_maps = _prep_inputs(x, Wk, Wq, Wv)
    nc = _build_nc()
    trace = bool(int(os.environ.get("KERNEL_TRACE", "0")))
    res = run_bass_kernel_spmd(
        nc, in_maps, core_ids=list(range(N_CORES)), trace=trace
    )
    LAST_RESULT = res
    return _assemble_output(res.results)
